# revision 11
# baseline (speedup 1.0000x reference)
"""Causal self-attention (B=1, T=4096, C=1024, H=16, D=64) on 8 NeuronCores.

Sharding: tensor-parallel over heads. Core i handles heads (2i, 2i+1):
it computes q/k/v projections for its 128 qkv columns, attention for its
2 heads, and a partial output projection (rank-128 slice of the
contraction). The host sums the 8 partial outputs and adds b_proj.

v3 layout/scheduling notes:
  - scores for a 128-k block are FOUR K=64/M=64 matmuls in the four PE
    quadrants (tile positions auto-derived), both heads concurrent.
  - per q-tile the k-blocks are visited diagonal-first so the mask
    multiplies (gpsimd/DVE) overlap the long unmasked run instead of
    sitting on the tile-end critical path.
  - per block the (sem-waiting) av matmuls are emitted BEFORE the next
    scores quadruple so the quadrant matmuls sit adjacent in the PE
    queue and launch concurrently.
  - av result is normalized BEFORE the projection: denominators ->
    fast reciprocal -> K=1 broadcast matmuls -> u = av * rr; the
    projection then contracts both heads in one K=128 group. Tail work
    is spread over fixed block slots of the next tile.
  - ACT does exp only. qkv tiles are emitted as half-units through a
    work queue into spare block slots; inputs arrive as 4 packed DMAs
    plus j-major xT slices so the first matmul starts ~5us in.
"""

import sys

if "/opt/trn_rl_repo" not in sys.path:
    sys.path.insert(0, "/opt/trn_rl_repo")

import numpy as np
import ml_dtypes

T = 4096
C = 1024
H = 16
D = 64
NCORES = 8
HPC = H // NCORES  # heads per core = 2
QT = 512  # q-tile width
NQT = T // QT  # 8
KB = 128  # k-block
NKB = T // KB  # 32
BF16 = ml_dtypes.bfloat16
OUT_BF16 = True  # partial outputs in bf16 (summed in f32 on host)

_COMPILED = {}


def _build_nc(with_bias=True):
    import concourse.tile as tile
    from concourse import bacc, mybir

    F32 = mybir.dt.float32
    BF = mybir.dt.bfloat16
    ODT = BF if OUT_BF16 else F32
    Exp = mybir.ActivationFunctionType.Exp

    nc = bacc.Bacc("TRN2", target_bir_lowering=False, debug=False,
                   num_devices=NCORES)

    def din(name, shape, dt=BF):
        if dt is None:
            dt = F32
        return nc.dram_tensor(name, shape, dt, kind="ExternalInput").ap()

    xT = din("xT", [C, T])                   # x transposed, bf16
    wpack = din("wpack", [128, 4 * C + 64])  # wq|wk|wv|wp|ident
    spack = din("spack", [1, 3 * 128 + QT])  # bq|bk|bv|ones
    mpack = din("mpack", [128, 8 * QT])      # mask0..3 (each [128, 2QT])
    onesf = din("onesf", [1, 128], dt=None)  # f32 ones (broadcast lhsT)
    out = nc.dram_tensor("out", [T, C], ODT, kind="ExternalOutput").ap()

    with tile.TileContext(nc) as tc:
        with (
            tc.tile_pool(name="const", bufs=1) as cpool,
            tc.tile_pool(name="qkv", bufs=1) as qkvpool,
            tc.tile_pool(name="exp", bufs=6) as epool,
            tc.tile_pool(name="small", bufs=2) as spool,
            tc.tile_pool(name="ostage", bufs=3) as opool,
            tc.tile_pool(name="ps_sc", bufs=2, space="PSUM") as ps_sc,
            tc.tile_pool(name="ps_sm", bufs=2, space="PSUM") as ps_sm,
            tc.tile_pool(name="ps_avA", bufs=1, space="PSUM") as ps_avA,
            tc.tile_pool(name="ps_avB", bufs=1, space="PSUM") as ps_avB,
        ):
            # ---- resident inputs: 4 packed DMAs + j-major xT slices.
            # scalar ring: weights/smalls/masks; sync ring: xT (j-major)
            # then nothing (outputs go to gpsimd's SWDGE). ----
            wall = cpool.tile([128, 4 * C + 64], BF, tag="wpack")
            nc.scalar.dma_start(wall[:], wpack[:])
            w_sb = {nm: wall[:, k * C:(k + 1) * C]
                    for k, nm in enumerate(("wq", "wk", "wv", "wp"))}
            ident_sb = wall[:, 4 * C:4 * C + 64]
            sall = cpool.tile([1, 3 * 128 + QT], BF, tag="spack")
            nc.scalar.dma_start(sall[:], spack[:])
            b_sb = {nm: sall[:, k * 128:(k + 1) * 128]
                    for k, nm in enumerate(("bq", "bk", "bv"))}
            ones_sb = sall[:, 3 * 128:]
            onesf_sb = cpool.tile([1, 128], F32, tag="onesf")
            nc.scalar.dma_start(onesf_sb[:], onesf[:])
            mall = cpool.tile([128, 8 * QT], BF, tag="mpack")
            nc.scalar.dma_start(mall[:], mpack[:])
            m_sb = [mall[:, d * 2 * QT:(d + 1) * 2 * QT] for d in range(4)]

            xT_sb = cpool.tile([128, 8, T], BF, tag="xT")
            xTv = xT.rearrange("(k p) t -> p k t", p=128)
            for j0, j1 in ((0, 1), (1, 2), (2, 4), (4, 6), (6, 8)):
                nc.sync.dma_start(
                    xT_sb[:, :, j0 * QT:j1 * QT],
                    xTv[:, :, j0 * QT:j1 * QT])

            qT_sb = qkvpool.tile([128, T], BF, tag="qT")
            kT_sb = qkvpool.tile([128, T], BF, tag="kT")
            vT_sb = qkvpool.tile([128, T], BF, tag="vT")
            vstore = []
            for h in range(2):
                vs = qkvpool.tile([128, NKB, 65], BF, tag=f"vst{h}",
                                  name=f"vst{h}")
                nc.gpsimd.memset(vs[:, :, 64], 1.0)
                vstore.append(vs)

            # ---- qkv work units (half-units keep PE bursts ~1us) ----
            def qkv_halves(wt, bias, dst, j):
                ps_box = []

                def first():
                    ps = ps_sm.tile([128, QT], F32, tag="ps", name="psqkv")
                    ps_box.append(ps)
                    for c0 in range(4):
                        nc.tensor.matmul(
                            ps[:],
                            lhsT=w_sb[wt][:, c0 * 128:(c0 + 1) * 128],
                            rhs=xT_sb[:, c0, j * QT:(j + 1) * QT],
                            start=(c0 == 0), stop=False)

                def second():
                    ps = ps_box[0]
                    for c0 in range(4, 8):
                        nc.tensor.matmul(
                            ps[:],
                            lhsT=w_sb[wt][:, c0 * 128:(c0 + 1) * 128],
                            rhs=xT_sb[:, c0, j * QT:(j + 1) * QT],
                            start=False,
                            stop=(not with_bias and c0 == 7))
                    if with_bias:
                        nc.tensor.matmul(ps[:], lhsT=b_sb[bias][:],
                                         rhs=ones_sb[:], start=False,
                                         stop=True)
                    nc.vector.tensor_copy(dst[:, j * QT:(j + 1) * QT], ps[:])

                return [first, second]

            def vprime_unit(blk):
                for h in range(2):
                    pt = ps_sm.tile([128, 64], BF, tag="ps", name="pt")
                    nc.tensor.transpose(
                        pt[:, 0:64],
                        vT_sb[h * 64:(h + 1) * 64, blk * 128:(blk + 1) * 128],
                        ident_sb[h * 64:(h + 1) * 64, :])
                    nc.vector.tensor_copy(vstore[h][:, blk, 0:64],
                                          pt[:, 0:64])

            def qkv_tile_items(j):
                items = []
                items += qkv_halves("wv", "bv", vT_sb, j)
                items += qkv_halves("wk", "bk", kT_sb, j)
                items += qkv_halves("wq", "bq", qT_sb, j)
                for c in range(4):
                    items.append(lambda blk=4 * j + c: vprime_unit(blk))
                return items

            # ---- attention pieces ----
            def emit_scores(i, b):
                """scores block b (both heads, 4 PE quadrants) -> exp/mask."""
                ps = ps_sc.tile([128, 2 * QT], F32, tag="sc", name="sc")
                for h in range(2):
                    hs = slice(h * 64, (h + 1) * 64)
                    for half in range(2):
                        k0 = b * 128 + half * 64
                        nc.tensor.matmul(
                            ps[half * 64:half * 64 + 64,
                               h * QT:(h + 1) * QT],
                            lhsT=kT_sb[hs, k0:k0 + 64],
                            rhs=qT_sb[hs, i * QT:(i + 1) * QT],
                            start=True, stop=True)
                et = epool.tile([128, 2 * QT], BF, tag="exp", name="et")
                d = b - 4 * i  # diagonal-block offset /128
                if d in (2, 3):
                    off = 128 * d
                    etv = et[:].rearrange("p (h q) -> p h q", h=2)
                    psv = ps[:].rearrange("p (h q) -> p h q", h=2)
                    mv = m_sb[d].rearrange("p (h q) -> p h q", h=2)
                    nc.gpsimd.memset(etv[:, :, 0:off], 0.0)
                    nc.scalar.activation(etv[:, :, off:QT], psv[:, :, off:QT],
                                         Exp, scale=0.125)
                    nc.vector.tensor_mul(etv[:, :, off:QT], etv[:, :, off:QT],
                                         mv[:, :, off:QT])
                else:
                    nc.scalar.activation(et[:], ps[:], Exp, scale=0.125)
                    if d in (0, 1):
                        nc.gpsimd.tensor_mul(et[:], et[:], m_sb[d])
                return et

            def emit_av(i, b, et, avA, avB, first, last):
                for h, av in ((0, avA), (1, avB)):
                    nc.tensor.matmul(
                        av[0:65, :],
                        lhsT=vstore[h][:, b, :],
                        rhs=et[:, h * QT:(h + 1) * QT],
                        start=first, stop=last)

            def tail_sums_a(i, avA, avB):
                """denominator rows -> fast reciprocal (DVE only)."""
                s2 = spool.tile([1, 2 * QT], F32, tag="s2", name="s2")
                nc.vector.tensor_copy(s2[0:1, 0:QT], avA[64:65, :])
                nc.vector.tensor_copy(s2[0:1, QT:2 * QT], avB[64:65, :])
                r2 = spool.tile([1, 2 * QT], F32, tag="r2", name="r2")
                nc.vector.reciprocal_approx_fast(r2[:], s2[:])
                return r2

            def tail_sums_b(i, avA, avB, r2):
                """broadcast reciprocals -> normalized u (bf16)."""
                rp = ps_sm.tile([128, QT], F32, tag="ps", name="rp")
                for h in range(2):
                    nc.tensor.matmul(rp[h * 64:(h + 1) * 64, :],
                                     lhsT=onesf_sb[0:1, 0:64],
                                     rhs=r2[0:1, h * QT:(h + 1) * QT],
                                     start=True, stop=True)
                rr = spool.tile([128, QT], F32, tag="rr", name="rr")
                nc.vector.tensor_copy(rr[:], rp[:])
                u = spool.tile([128, QT], BF, tag="u", name="u")
                nc.vector.tensor_mul(u[0:64, :], avA[0:64, :], rr[0:64, :])
                nc.vector.tensor_mul(u[64:128, :], avB[0:64, :],
                                     rr[64:128, :])
                return u

            def tail_proj_chunk(i, u, cchunk):
                qs = slice(cchunk * 128, (cchunk + 1) * 128)
                ost = opool.tile([128, C], ODT, tag="ost", name="ost")
                for chalf in range(2):
                    cs = slice(chalf * QT, (chalf + 1) * QT)
                    pp = ps_sm.tile([128, QT], F32, tag="ps", name="pp")
                    nc.tensor.matmul(pp[:], lhsT=u[:, qs],
                                     rhs=w_sb["wp"][:, cs],
                                     start=True, stop=True)
                    nc.vector.tensor_copy(ost[:, cs], pp[:])
                row = i * QT + cchunk * 128
                nc.gpsimd.dma_start(out[row:row + 128, :], ost[:])

            # ---- main loop ----
            workq = []  # (deadline_tile, fn): emit before attention(deadline)

            def drain(upto):
                while workq and workq[0][0] <= upto:
                    workq.pop(0)[1]()

            for fn in qkv_tile_items(0):
                fn()
            for fn in qkv_tile_items(1):
                fn()
            for j in range(2, NQT):
                for fn in qkv_tile_items(j):
                    workq.append((j, fn))

            pend = None  # [i, avA, avB, r2, u] tail state
            for i in range(NQT):
                drain(i)
                avA = ps_avA.tile([128, QT], F32, tag="avA", name="avA")
                avB = ps_avB.tile([128, QT], F32, tag="avB", name="avB")
                nblk = 4 * (i + 1)
                # diagonal blocks first, then ascending far blocks
                order = list(range(nblk - 1, max(nblk - 5, -1), -1)) + \
                    list(range(0, max(nblk - 4, 0)))
                pend_av = None  # (b, et, first)
                for slot, b in enumerate(order):
                    if pend_av is not None:
                        emit_av(i, pend_av[0], pend_av[1], avA, avB,
                                pend_av[2], False)
                    et = emit_scores(i, b)
                    if pend is not None and slot == 0:
                        with tc.high_priority():
                            pend[3] = tail_sums_a(pend[0], pend[1], pend[2])
                    elif pend is not None and slot == 2:
                        with tc.high_priority():
                            pend[4] = tail_sums_b(pend[0], pend[1], pend[2],
                                                  pend[3])
                    elif pend is not None and 4 <= slot <= 7:
                        tail_proj_chunk(pend[0], pend[4], slot - 4)
                        if slot == 7:
                            pend = None
                    elif workq:
                        workq.pop(0)[1]()
                    pend_av = (b, et, slot == 0)
                emit_av(i, pend_av[0], pend_av[1], avA, avB,
                        pend_av[2], True)
                pend = [i, avA, avB, None, None]
            # final tail
            pi, pA, pB = pend[0], pend[1], pend[2]
            r2 = tail_sums_a(pi, pA, pB)
            u = tail_sums_b(pi, pA, pB, r2)
            for cc in range(4):
                tail_proj_chunk(pi, u, cc)

    nc.compile()
    return nc


def _causal_mask(d):
    kp = np.arange(128)[:, None]
    qf = np.arange(QT)[None, :]
    return ((kp + d) <= qf).astype(BF16)


def _prep_inputs(x, w_qkv, b_qkv, w_proj):
    """Build the 8 per-core input maps (host-side shard + pack)."""
    xT = np.ascontiguousarray(x.reshape(T, C).T).astype(BF16)
    mpack = np.concatenate(
        [np.concatenate([_causal_mask(128 * d)] * 2, axis=1)
         for d in range(4)], axis=1)
    ident = np.zeros((128, 64), dtype=BF16)
    ident[np.arange(128), np.arange(128) % 64] = 1
    onesf = np.ones((1, 128), dtype=np.float32)

    def pack_w(wcols):  # [C, 128] -> [128, C] chunk-packed for SBUF
        return np.ascontiguousarray(
            wcols.reshape(8, 128, 128).transpose(1, 0, 2).reshape(128, C)
        ).astype(BF16)

    in_maps = []
    for core in range(NCORES):
        h0 = core * HPC
        cols = slice(h0 * D, (h0 + HPC) * D)  # 128 cols for this core
        wq = pack_w(w_qkv[:, :C][:, cols])
        wk = pack_w(w_qkv[:, C:2 * C][:, cols])
        wv = pack_w(w_qkv[:, 2 * C:][:, cols])
        wp = np.ascontiguousarray(w_proj[cols, :]).astype(BF16)
        wpack = np.concatenate([wq, wk, wv, wp, ident], axis=1)
        spack = np.concatenate(
            [b_qkv[:C][cols].reshape(1, 128),
             b_qkv[C:2 * C][cols].reshape(1, 128),
             b_qkv[2 * C:][cols].reshape(1, 128),
             np.ones((1, QT))], axis=1).astype(BF16)
        m = {
            "xT": xT,
            "wpack": np.ascontiguousarray(wpack),
            "spack": np.ascontiguousarray(spack),
            "mpack": np.ascontiguousarray(mpack),
            "onesf": onesf,
        }
        in_maps.append(m)
    return in_maps


def _get_compiled(with_bias=True):
    if with_bias not in _COMPILED:
        _COMPILED[with_bias] = _build_nc(with_bias=with_bias)
    return _COMPILED[with_bias]


def run_on_device(in_maps, with_bias=True, **kwargs):
    from concourse.bass_utils import run_bass_kernel_spmd

    nc = _get_compiled(with_bias)
    return run_bass_kernel_spmd(nc, in_maps, core_ids=list(range(NCORES)),
                                **kwargs)


def kernel(x, w_qkv, b_qkv, w_proj, b_proj, **run_kwargs):
    x = np.asarray(x, dtype=np.float32)
    w_qkv = np.asarray(w_qkv, dtype=np.float32)
    b_qkv = np.asarray(b_qkv, dtype=np.float32)
    w_proj = np.asarray(w_proj, dtype=np.float32)
    b_proj = np.asarray(b_proj, dtype=np.float32)

    in_maps = _prep_inputs(x, w_qkv, b_qkv, w_proj)
    with_bias = bool(np.any(b_qkv))
    res = run_on_device(in_maps, with_bias=with_bias, **run_kwargs)
    acc = np.zeros((T, C), dtype=np.float32)
    for core in range(NCORES):
        acc += np.asarray(res.results[core]["out"], dtype=np.float32)
    acc += b_proj[None, :]
    out = acc.reshape(1, T, C)
    kernel.last_results = res
    return out


# revision 15
# speedup vs baseline: 1.0057x; 1.0057x over previous
"""Causal self-attention (B=1, T=4096, C=1024, H=16, D=64) on 8 NeuronCores.

Sharding: tensor-parallel over heads. Core i handles heads (2i, 2i+1):
it computes q/k/v projections for its 128 qkv columns, attention for its
2 heads, and a partial output projection (rank-128 slice of the
contraction). The host sums the 8 partial outputs and adds b_proj.

v3 layout/scheduling notes:
  - scores for a 128-k block are FOUR K=64/M=64 matmuls in the four PE
    quadrants (tile positions auto-derived), both heads concurrent.
  - per q-tile the k-blocks are visited diagonal-first so the mask
    multiplies (gpsimd/DVE) overlap the long unmasked run instead of
    sitting on the tile-end critical path.
  - per block the (sem-waiting) av matmuls are emitted BEFORE the next
    scores quadruple so the quadrant matmuls sit adjacent in the PE
    queue and launch concurrently.
  - av result is normalized BEFORE the projection: denominators ->
    fast reciprocal -> K=1 broadcast matmuls -> u = av * rr; the
    projection then contracts both heads in one K=128 group. Tail work
    is spread over fixed block slots of the next tile.
  - ACT does exp only. qkv tiles are emitted as half-units through a
    work queue into spare block slots; inputs arrive as 4 packed DMAs
    plus j-major xT slices so the first matmul starts ~5us in.
"""

import sys

if "/opt/trn_rl_repo" not in sys.path:
    sys.path.insert(0, "/opt/trn_rl_repo")

import numpy as np
import ml_dtypes

T = 4096
C = 1024
H = 16
D = 64
NCORES = 8
HPC = H // NCORES  # heads per core = 2
QT = 512  # q-tile width
NQT = T // QT  # 8
KB = 128  # k-block
NKB = T // KB  # 32
BF16 = ml_dtypes.bfloat16
OUT_BF16 = True  # partial outputs in bf16 (summed in f32 on host)

_COMPILED = {}


def _build_nc(with_bias=True):
    import concourse.tile as tile
    from concourse import bacc, mybir

    F32 = mybir.dt.float32
    BF = mybir.dt.bfloat16
    ODT = BF if OUT_BF16 else F32
    Exp = mybir.ActivationFunctionType.Exp

    nc = bacc.Bacc("TRN2", target_bir_lowering=False, debug=False,
                   num_devices=NCORES)

    def din(name, shape, dt=BF):
        if dt is None:
            dt = F32
        return nc.dram_tensor(name, shape, dt, kind="ExternalInput").ap()

    xT = din("xT", [C, T])                   # x transposed, bf16
    wpack = din("wpack", [128, 4 * C + 64])  # wq|wk|wv|wp|ident
    spack = din("spack", [1, 3 * 128 + QT])  # bq|bk|bv|ones
    mpack = din("mpack", [128, 8 * QT])      # mask0..3 (each [128, 2QT])
    onesf = din("onesf", [1, 128], dt=None)  # f32 ones (broadcast lhsT)
    out = nc.dram_tensor("out", [T, C], ODT, kind="ExternalOutput").ap()

    with tile.TileContext(nc) as tc:
        with (
            tc.tile_pool(name="const", bufs=1) as cpool,
            tc.tile_pool(name="qkv", bufs=1) as qkvpool,
            tc.tile_pool(name="exp", bufs=6) as epool,
            tc.tile_pool(name="small", bufs=2) as spool,
            tc.tile_pool(name="ostage", bufs=3) as opool,
            tc.tile_pool(name="ps_sc", bufs=2, space="PSUM") as ps_sc,
            tc.tile_pool(name="ps_sm", bufs=1, space="PSUM") as ps_sm,
            tc.tile_pool(name="ps_qk", bufs=1, space="PSUM") as ps_qk,
            tc.tile_pool(name="ps_avA", bufs=1, space="PSUM") as ps_avA,
            tc.tile_pool(name="ps_avB", bufs=1, space="PSUM") as ps_avB,
        ):
            # ---- resident inputs: 4 packed DMAs + j-major xT slices.
            # scalar ring: weights/smalls/masks; sync ring: xT (j-major)
            # then nothing (outputs go to gpsimd's SWDGE). ----
            wall = cpool.tile([128, 4 * C + 64], BF, tag="wpack")
            nc.scalar.dma_start(wall[:], wpack[:])
            w_sb = {nm: wall[:, k * C:(k + 1) * C]
                    for k, nm in enumerate(("wq", "wk", "wv", "wp"))}
            ident_sb = wall[:, 4 * C:4 * C + 64]
            sall = cpool.tile([1, 3 * 128 + QT], BF, tag="spack")
            nc.scalar.dma_start(sall[:], spack[:])
            b_sb = {nm: sall[:, k * 128:(k + 1) * 128]
                    for k, nm in enumerate(("bq", "bk", "bv"))}
            ones_sb = sall[:, 3 * 128:]
            onesf_sb = cpool.tile([1, 128], F32, tag="onesf")
            nc.scalar.dma_start(onesf_sb[:], onesf[:])
            mall = cpool.tile([128, 8 * QT], BF, tag="mpack")
            nc.scalar.dma_start(mall[:], mpack[:])
            m_sb = [mall[:, d * 2 * QT:(d + 1) * 2 * QT] for d in range(4)]

            xT_sb = cpool.tile([128, 8, T], BF, tag="xT")
            xTv = xT.rearrange("(k p) t -> p k t", p=128)
            for j0, j1 in ((0, 1), (1, 2), (2, 4), (4, 6), (6, 8)):
                nc.sync.dma_start(
                    xT_sb[:, :, j0 * QT:j1 * QT],
                    xTv[:, :, j0 * QT:j1 * QT])

            qT_sb = qkvpool.tile([128, T], BF, tag="qT")
            kT_sb = qkvpool.tile([128, T], BF, tag="kT")
            vT_sb = qkvpool.tile([128, T], BF, tag="vT")
            vstore = []
            for h in range(2):
                vs = qkvpool.tile([128, NKB, 65], BF, tag=f"vst{h}",
                                  name=f"vst{h}")
                nc.gpsimd.memset(vs[:, :, 64], 1.0)
                vstore.append(vs)

            # ---- qkv work units ----
            def qkv_unit(wt, bias, dst, j):
                ps = ps_qk.tile([128, QT], F32, tag="ps", name="psqkv")
                for c0 in range(8):
                    nc.tensor.matmul(
                        ps[:],
                        lhsT=w_sb[wt][:, c0 * 128:(c0 + 1) * 128],
                        rhs=xT_sb[:, c0, j * QT:(j + 1) * QT],
                        start=(c0 == 0),
                        stop=(not with_bias and c0 == 7))
                if with_bias:
                    nc.tensor.matmul(ps[:], lhsT=b_sb[bias],
                                     rhs=ones_sb, start=False, stop=True)
                nc.vector.tensor_copy(dst[:, j * QT:(j + 1) * QT], ps[:])

            def vprime_unit(blk):
                for h in range(2):
                    pt = ps_qk.tile([128, 64], BF, tag="ps", name="pt")
                    nc.tensor.transpose(
                        pt[:, 0:64],
                        vT_sb[h * 64:(h + 1) * 64, blk * 128:(blk + 1) * 128],
                        ident_sb[h * 64:(h + 1) * 64, :])
                    nc.vector.tensor_copy(vstore[h][:, blk, 0:64],
                                          pt[:, 0:64])

            def qkv_tile(j):
                qkv_unit("wv", "bv", vT_sb, j)
                qkv_unit("wk", "bk", kT_sb, j)
                qkv_unit("wq", "bq", qT_sb, j)
                for c in range(4):
                    vprime_unit(4 * j + c)

            # ---- attention pieces ----
            def emit_scores(i, b):
                """scores block b (both heads, 4 PE quadrants) -> exp/mask."""
                ps = ps_sc.tile([128, 2 * QT], F32, tag="sc", name="sc")
                with tc.high_priority():
                    for h in range(2):
                        hs = slice(h * 64, (h + 1) * 64)
                        for half in range(2):
                            k0 = b * 128 + half * 64
                            nc.tensor.matmul(
                                ps[half * 64:half * 64 + 64,
                                   h * QT:(h + 1) * QT],
                                lhsT=kT_sb[hs, k0:k0 + 64],
                                rhs=qT_sb[hs, i * QT:(i + 1) * QT],
                                start=True, stop=True)
                et = epool.tile([128, 2 * QT], BF, tag="exp", name="et")
                d = b - 4 * i  # diagonal-block offset /128
                if d in (2, 3):
                    off = 128 * d
                    etv = et[:].rearrange("p (h q) -> p h q", h=2)
                    psv = ps[:].rearrange("p (h q) -> p h q", h=2)
                    mv = m_sb[d].rearrange("p (h q) -> p h q", h=2)
                    nc.gpsimd.memset(etv[:, :, 0:off], 0.0)
                    nc.scalar.activation(etv[:, :, off:QT], psv[:, :, off:QT],
                                         Exp, scale=0.125)
                    nc.vector.tensor_mul(etv[:, :, off:QT], etv[:, :, off:QT],
                                         mv[:, :, off:QT])
                else:
                    nc.scalar.activation(et[:], ps[:], Exp, scale=0.125)
                    if d in (0, 1):
                        nc.gpsimd.tensor_mul(et[:], et[:], m_sb[d])
                return et

            def emit_av(i, b, et, avA, avB, first, last):
                for h, av in ((0, avA), (1, avB)):
                    nc.tensor.matmul(
                        av[0:65, :],
                        lhsT=vstore[h][:, b, :],
                        rhs=et[:, h * QT:(h + 1) * QT],
                        start=first, stop=last)

            def tail_sums_a(i, avA, avB):
                """denominator rows -> fast reciprocal (DVE only)."""
                s2 = spool.tile([1, 2 * QT], F32, tag="s2", name="s2")
                nc.vector.tensor_copy(s2[0:1, 0:QT], avA[64:65, :])
                nc.vector.tensor_copy(s2[0:1, QT:2 * QT], avB[64:65, :])
                r2 = spool.tile([1, 2 * QT], F32, tag="r2", name="r2")
                nc.vector.reciprocal_approx_fast(r2[:], s2[:])
                return r2

            def tail_sums_b(i, avA, avB, r2):
                """broadcast reciprocals -> normalized u (bf16)."""
                rp = ps_sm.tile([128, QT], F32, tag="ps", name="rp")
                for h in range(2):
                    nc.tensor.matmul(rp[h * 64:(h + 1) * 64, :],
                                     lhsT=onesf_sb[0:1, 0:64],
                                     rhs=r2[0:1, h * QT:(h + 1) * QT],
                                     start=True, stop=True)
                rr = spool.tile([128, QT], F32, tag="rr", name="rr")
                nc.vector.tensor_copy(rr[:], rp[:])
                u = spool.tile([128, QT], BF, tag="u", name="u")
                nc.vector.tensor_mul(u[0:64, :], avA[0:64, :], rr[0:64, :])
                nc.vector.tensor_mul(u[64:128, :], avB[0:64, :],
                                     rr[64:128, :])
                return u

            def tail_proj_chunk(i, u, cchunk):
                qs = slice(cchunk * 128, (cchunk + 1) * 128)
                ost = opool.tile([128, C], ODT, tag="ost", name="ost")
                for chalf in range(2):
                    cs = slice(chalf * QT, (chalf + 1) * QT)
                    pp = ps_sm.tile([128, QT], F32, tag="ps", name="pp")
                    nc.tensor.matmul(pp[:], lhsT=u[:, qs],
                                     rhs=w_sb["wp"][:, cs],
                                     start=True, stop=True)
                    nc.vector.tensor_copy(ost[:, cs], pp[:])
                row = i * QT + cchunk * 128
                nc.gpsimd.dma_start(out[row:row + 128, :], ost[:])

            # ---- main loop. qkv tiles 0/1 gate attention start; tiles
            # 2..7 are emitted LAST with high priority numbers so the
            # list scheduler uses them as PE idle-time filler, pulled
            # forward only by the data deps of the attention stream. ----
            qkv_tile(0)
            qkv_tile(1)

            for i in range(NQT):
                avA = ps_avA.tile([128, QT], F32, tag="avA", name="avA")
                avB = ps_avB.tile([128, QT], F32, tag="avB", name="avB")
                nblk = 4 * (i + 1)
                # diagonal blocks first, then ascending far blocks
                order = list(range(nblk - 1, max(nblk - 5, -1), -1)) + \
                    list(range(0, max(nblk - 4, 0)))
                pend_av = None  # (b, et, first)
                for slot, b in enumerate(order):
                    et = emit_scores(i, b)
                    if pend_av is not None:
                        emit_av(i, pend_av[0], pend_av[1], avA, avB,
                                pend_av[2], False)
                    pend_av = (b, et, slot == 0)
                emit_av(i, pend_av[0], pend_av[1], avA, avB,
                        pend_av[2], True)
                r2 = tail_sums_a(i, avA, avB)
                u = tail_sums_b(i, avA, avB, r2)
                for cc in range(4):
                    tail_proj_chunk(i, u, cc)

            for j in range(2, NQT):
                qkv_tile(j)

    nc.compile()
    return nc


def _causal_mask(d):
    kp = np.arange(128)[:, None]
    qf = np.arange(QT)[None, :]
    return ((kp + d) <= qf).astype(BF16)


def _prep_inputs(x, w_qkv, b_qkv, w_proj):
    """Build the 8 per-core input maps (host-side shard + pack)."""
    xT = np.ascontiguousarray(x.reshape(T, C).T).astype(BF16)
    mpack = np.concatenate(
        [np.concatenate([_causal_mask(128 * d)] * 2, axis=1)
         for d in range(4)], axis=1)
    ident = np.zeros((128, 64), dtype=BF16)
    ident[np.arange(128), np.arange(128) % 64] = 1
    onesf = np.ones((1, 128), dtype=np.float32)

    def pack_w(wcols):  # [C, 128] -> [128, C] chunk-packed for SBUF
        return np.ascontiguousarray(
            wcols.reshape(8, 128, 128).transpose(1, 0, 2).reshape(128, C)
        ).astype(BF16)

    in_maps = []
    for core in range(NCORES):
        h0 = core * HPC
        cols = slice(h0 * D, (h0 + HPC) * D)  # 128 cols for this core
        wq = pack_w(w_qkv[:, :C][:, cols])
        wk = pack_w(w_qkv[:, C:2 * C][:, cols])
        wv = pack_w(w_qkv[:, 2 * C:][:, cols])
        wp = np.ascontiguousarray(w_proj[cols, :]).astype(BF16)
        wpack = np.concatenate([wq, wk, wv, wp, ident], axis=1)
        spack = np.concatenate(
            [b_qkv[:C][cols].reshape(1, 128),
             b_qkv[C:2 * C][cols].reshape(1, 128),
             b_qkv[2 * C:][cols].reshape(1, 128),
             np.ones((1, QT))], axis=1).astype(BF16)
        m = {
            "xT": xT,
            "wpack": np.ascontiguousarray(wpack),
            "spack": np.ascontiguousarray(spack),
            "mpack": np.ascontiguousarray(mpack),
            "onesf": onesf,
        }
        in_maps.append(m)
    return in_maps


def _get_compiled(with_bias=True):
    if with_bias not in _COMPILED:
        _COMPILED[with_bias] = _build_nc(with_bias=with_bias)
    return _COMPILED[with_bias]


def run_on_device(in_maps, with_bias=True, **kwargs):
    from concourse.bass_utils import run_bass_kernel_spmd

    nc = _get_compiled(with_bias)
    return run_bass_kernel_spmd(nc, in_maps, core_ids=list(range(NCORES)),
                                **kwargs)


def kernel(x, w_qkv, b_qkv, w_proj, b_proj, **run_kwargs):
    x = np.asarray(x, dtype=np.float32)
    w_qkv = np.asarray(w_qkv, dtype=np.float32)
    b_qkv = np.asarray(b_qkv, dtype=np.float32)
    w_proj = np.asarray(w_proj, dtype=np.float32)
    b_proj = np.asarray(b_proj, dtype=np.float32)

    in_maps = _prep_inputs(x, w_qkv, b_qkv, w_proj)
    with_bias = bool(np.any(b_qkv))
    res = run_on_device(in_maps, with_bias=with_bias, **run_kwargs)
    acc = np.zeros((T, C), dtype=np.float32)
    for core in range(NCORES):
        acc += np.asarray(res.results[core]["out"], dtype=np.float32)
    acc += b_proj[None, :]
    out = acc.reshape(1, T, C)
    kernel.last_results = res
    return out


# revision 21
# speedup vs baseline: 1.0324x; 1.0265x over previous
"""Causal self-attention (B=1, T=4096, C=1024, H=16, D=64) on 8 NeuronCores.

Sharding: tensor-parallel over heads. Core i handles heads (2i, 2i+1):
it computes q/k/v projections for its 128 qkv columns, attention for its
2 heads, and a partial output projection (rank-128 slice of the
contraction). The host sums the 8 partial outputs and adds b_proj.

v3 layout/scheduling notes:
  - scores for a 128-k block are FOUR K=64/M=64 matmuls in the four PE
    quadrants (tile positions auto-derived), both heads concurrent.
  - per q-tile the k-blocks are visited diagonal-first so the mask
    multiplies (gpsimd/DVE) overlap the long unmasked run instead of
    sitting on the tile-end critical path.
  - per block the (sem-waiting) av matmuls are emitted BEFORE the next
    scores quadruple so the quadrant matmuls sit adjacent in the PE
    queue and launch concurrently.
  - av result is normalized BEFORE the projection: denominators ->
    fast reciprocal -> K=1 broadcast matmuls -> u = av * rr; the
    projection then contracts both heads in one K=128 group. Tail work
    is spread over fixed block slots of the next tile.
  - ACT does exp only. qkv tiles are emitted as half-units through a
    work queue into spare block slots; inputs arrive as 4 packed DMAs
    plus j-major xT slices so the first matmul starts ~5us in.
"""

import sys

if "/opt/trn_rl_repo" not in sys.path:
    sys.path.insert(0, "/opt/trn_rl_repo")

import numpy as np
import ml_dtypes

T = 4096
C = 1024
H = 16
D = 64
NCORES = 8
HPC = H // NCORES  # heads per core = 2
QT = 512  # q-tile width
NQT = T // QT  # 8
KB = 128  # k-block
NKB = T // KB  # 32
BF16 = ml_dtypes.bfloat16
OUT_BF16 = True  # partial outputs in bf16 (summed in f32 on host)

_COMPILED = {}


def _build_nc(with_bias=True):
    import concourse.tile as tile
    from concourse import bacc, mybir

    F32 = mybir.dt.float32
    BF = mybir.dt.bfloat16
    ODT = BF if OUT_BF16 else F32
    Exp = mybir.ActivationFunctionType.Exp

    nc = bacc.Bacc("TRN2", target_bir_lowering=False, debug=False,
                   num_devices=NCORES)

    def din(name, shape, dt=BF):
        if dt is None:
            dt = F32
        return nc.dram_tensor(name, shape, dt, kind="ExternalInput").ap()

    xT = din("xT", [C, T])                   # x transposed, bf16
    wpack = din("wpack", [128, 4 * C + 64])  # wq|wk|wv|wp|ident
    spack = din("spack", [1, 3 * 128 + QT])  # bq|bk|bv|ones
    mpack = din("mpack", [128, 8 * QT])      # mask0..3 (each [128, 2QT])
    onesf = din("onesf", [1, 128], dt=None)  # f32 ones (broadcast lhsT)
    out = nc.dram_tensor("out", [T, C], ODT, kind="ExternalOutput").ap()

    with tile.TileContext(nc) as tc:
        with (
            tc.tile_pool(name="const", bufs=1) as cpool,
            tc.tile_pool(name="qkv", bufs=1) as qkvpool,
            tc.tile_pool(name="exp", bufs=6) as epool,
            tc.tile_pool(name="small", bufs=2) as spool,
            tc.tile_pool(name="ostage", bufs=3) as opool,
            tc.tile_pool(name="ps_sc", bufs=2, space="PSUM") as ps_sc,
            tc.tile_pool(name="ps_qk", bufs=2, space="PSUM") as ps_qk,
            tc.tile_pool(name="ps_avA", bufs=1, space="PSUM") as ps_avA,
            tc.tile_pool(name="ps_avB", bufs=1, space="PSUM") as ps_avB,
        ):
            # ---- resident inputs: 4 packed DMAs + j-major xT slices.
            # scalar ring: weights/smalls/masks; sync ring: xT (j-major)
            # then nothing (outputs go to gpsimd's SWDGE). ----
            wall = cpool.tile([128, 4 * C + 64], BF, tag="wpack")
            nc.scalar.dma_start(wall[:], wpack[:])
            w_sb = {nm: wall[:, k * C:(k + 1) * C]
                    for k, nm in enumerate(("wq", "wk", "wv", "wp"))}
            ident_sb = wall[:, 4 * C:4 * C + 64]
            sall = cpool.tile([1, 3 * 128 + QT], BF, tag="spack")
            nc.scalar.dma_start(sall[:], spack[:])
            b_sb = {nm: sall[:, k * 128:(k + 1) * 128]
                    for k, nm in enumerate(("bq", "bk", "bv"))}
            ones_sb = sall[:, 3 * 128:]
            onesf_sb = cpool.tile([1, 128], F32, tag="onesf")
            nc.scalar.dma_start(onesf_sb[:], onesf[:])
            mall = cpool.tile([128, 8 * QT], BF, tag="mpack")
            nc.scalar.dma_start(mall[:], mpack[:])
            m_sb = [mall[:, d * 2 * QT:(d + 1) * 2 * QT] for d in range(4)]

            xT_sb = cpool.tile([128, 8, T], BF, tag="xT")
            xTv = xT.rearrange("(k p) t -> p k t", p=128)
            for j0, j1 in ((0, 1), (1, 2), (2, 4), (4, 6), (6, 8)):
                nc.sync.dma_start(
                    xT_sb[:, :, j0 * QT:j1 * QT],
                    xTv[:, :, j0 * QT:j1 * QT])

            qT_sb = qkvpool.tile([128, T], BF, tag="qT")
            kT_sb = qkvpool.tile([128, T], BF, tag="kT")
            vT_sb = qkvpool.tile([128, T], BF, tag="vT")
            vstore = []
            for h in range(2):
                vs = qkvpool.tile([128, NKB, 65], BF, tag=f"vst{h}",
                                  name=f"vst{h}")
                nc.gpsimd.memset(vs[:, :, 64], 1.0)
                vstore.append(vs)

            # ---- qkv work, as a stream of ~0.5us chunks so it can be
            # drip-fed into the attention blocks' PE slack ----
            def qkv_unit_chunks(wt, bias, dst, j):
                box = []

                def mms(c0lo, c0hi):
                    def fn():
                        if not box:
                            box.append(ps_qk.tile([128, QT], F32, tag="ps",
                                                  name="psqkv"))
                        ps = box[0]
                        for c0 in range(c0lo, c0hi):
                            nc.tensor.matmul(
                                ps[:],
                                lhsT=w_sb[wt][:, c0 * 128:(c0 + 1) * 128],
                                rhs=xT_sb[:, c0, j * QT:(j + 1) * QT],
                                start=(c0 == 0),
                                stop=(not with_bias and c0 == 7))
                        if c0hi == 8:
                            if with_bias:
                                nc.tensor.matmul(ps[:], lhsT=b_sb[bias],
                                                 rhs=ones_sb, start=False,
                                                 stop=True)
                            nc.vector.tensor_copy(
                                dst[:, j * QT:(j + 1) * QT], ps[:])
                    return fn

                return [mms(0, 2), mms(2, 4), mms(4, 6), mms(6, 8)]

            def vprime_unit(blk):
                for h in range(2):
                    pt = ps_qk.tile([128, 64], BF, tag="ps", name="pt")
                    nc.tensor.transpose(
                        pt[:, 0:64],
                        vT_sb[h * 64:(h + 1) * 64, blk * 128:(blk + 1) * 128],
                        ident_sb[h * 64:(h + 1) * 64, :])
                    nc.vector.tensor_copy(vstore[h][:, blk, 0:64],
                                          pt[:, 0:64])

            def qkv_tile_chunks(j):
                chunks = []
                chunks += qkv_unit_chunks("wv", "bv", vT_sb, j)
                chunks += qkv_unit_chunks("wk", "bk", kT_sb, j)
                chunks += qkv_unit_chunks("wq", "bq", qT_sb, j)
                for c in range(4):
                    chunks.append(lambda blk=4 * j + c: vprime_unit(blk))
                return chunks

            # ---- attention pieces ----
            def emit_scores(i, b):
                """scores block b (both heads, 4 PE quadrants) -> exp/mask."""
                ps = ps_sc.tile([128, 2 * QT], F32, tag="sc", name="sc")
                with tc.high_priority():
                    for h in range(2):
                        hs = slice(h * 64, (h + 1) * 64)
                        for half in range(2):
                            k0 = b * 128 + half * 64
                            nc.tensor.matmul(
                                ps[half * 64:half * 64 + 64,
                                   h * QT:(h + 1) * QT],
                                lhsT=kT_sb[hs, k0:k0 + 64],
                                rhs=qT_sb[hs, i * QT:(i + 1) * QT],
                                start=True, stop=True)
                et = epool.tile([128, 2 * QT], BF, tag="exp", name="et")
                d = b - 4 * i  # diagonal-block offset /128
                if d in (2, 3):
                    off = 128 * d
                    etv = et[:].rearrange("p (h q) -> p h q", h=2)
                    psv = ps[:].rearrange("p (h q) -> p h q", h=2)
                    mv = m_sb[d].rearrange("p (h q) -> p h q", h=2)
                    nc.gpsimd.memset(etv[:, :, 0:off], 0.0)
                    nc.scalar.activation(etv[:, :, off:QT], psv[:, :, off:QT],
                                         Exp, scale=0.125)
                    nc.vector.tensor_mul(etv[:, :, off:QT], etv[:, :, off:QT],
                                         mv[:, :, off:QT])
                else:
                    nc.scalar.activation(et[:], ps[:], Exp, scale=0.125)
                    if d in (0, 1):
                        nc.gpsimd.tensor_mul(et[:], et[:], m_sb[d])
                return et

            def emit_av(i, b, et, avA, avB, first, last):
                for h, av in ((0, avA), (1, avB)):
                    nc.tensor.matmul(
                        av[0:65, :],
                        lhsT=vstore[h][:, b, :],
                        rhs=et[:, h * QT:(h + 1) * QT],
                        start=first, stop=last)

            def tail_sums_a(i, avA, avB):
                """denominator rows -> fast reciprocal (DVE only)."""
                s2 = spool.tile([1, 2 * QT], F32, tag="s2", name="s2")
                nc.vector.tensor_copy(s2[0:1, 0:QT], avA[64:65, :])
                nc.vector.tensor_copy(s2[0:1, QT:2 * QT], avB[64:65, :])
                r2 = spool.tile([1, 2 * QT], F32, tag="r2", name="r2")
                nc.vector.reciprocal_approx_fast(r2[:], s2[:])
                return r2

            def tail_sums_b(i, avA, avB, r2):
                """broadcast reciprocals -> normalized u (bf16)."""
                rp = ps_sc.tile([128, QT], F32, tag="sc", name="rp")
                for h in range(2):
                    nc.tensor.matmul(rp[h * 64:(h + 1) * 64, :],
                                     lhsT=onesf_sb[0:1, 0:64],
                                     rhs=r2[0:1, h * QT:(h + 1) * QT],
                                     start=True, stop=True)
                rr = spool.tile([128, QT], F32, tag="rr", name="rr")
                nc.vector.tensor_copy(rr[:], rp[:])
                u = spool.tile([128, QT], BF, tag="u", name="u", bufs=NQT)
                nc.vector.tensor_mul(u[0:64, :], avA[0:64, :], rr[0:64, :])
                nc.vector.tensor_mul(u[64:128, :], avB[0:64, :],
                                     rr[64:128, :])
                return u

            def tail_proj_chunk(i, u, cchunk):
                qs = slice(cchunk * 128, (cchunk + 1) * 128)
                ost = opool.tile([128, C], ODT, tag="ost", name="ost")
                for chalf in range(2):
                    cs = slice(chalf * QT, (chalf + 1) * QT)
                    pp = ps_qk.tile([128, QT], F32, tag="ps", name="pp")
                    nc.tensor.matmul(pp[:], lhsT=u[:, qs],
                                     rhs=w_sb["wp"][:, cs],
                                     start=True, stop=True)
                    nc.vector.tensor_copy(ost[:, cs], pp[:])
                row = i * QT + cchunk * 128
                nc.gpsimd.dma_start(out[row:row + 128, :], ost[:])

            # ---- main loop. qkv tiles 0/1 up front; tiles 2..7 drip
            # into the attention blocks' PE slack, one ~0.5us chunk per
            # block, with a drain before the tile that needs them.
            # Projections run as end-emitted filler (deps pull them in
            # during the last tile's long exp stretch). ----
            for fn in qkv_tile_chunks(0):
                fn()
            for fn in qkv_tile_chunks(1):
                fn()
            workq = []  # (deadline_tile, chunk_fn)
            for j in range(2, NQT):
                for fn in qkv_tile_chunks(j):
                    workq.append((j, fn))

            us = {}
            for i in range(NQT):
                while workq and workq[0][0] <= i:
                    workq.pop(0)[1]()
                avA = ps_avA.tile([128, QT], F32, tag="avA", name="avA")
                avB = ps_avB.tile([128, QT], F32, tag="avB", name="avB")
                nblk = 4 * (i + 1)
                # diagonal blocks first, then ascending far blocks
                order = list(range(nblk - 1, max(nblk - 5, -1), -1)) + \
                    list(range(0, max(nblk - 4, 0)))
                pend_av = None  # (b, et, first)
                for slot, b in enumerate(order):
                    et = emit_scores(i, b)
                    if pend_av is not None:
                        emit_av(i, pend_av[0], pend_av[1], avA, avB,
                                pend_av[2], False)
                    if workq:
                        workq.pop(0)[1]()
                    pend_av = (b, et, slot == 0)
                emit_av(i, pend_av[0], pend_av[1], avA, avB,
                        pend_av[2], True)
                r2 = tail_sums_a(i, avA, avB)
                us[i] = tail_sums_b(i, avA, avB, r2)

            for i in range(NQT):
                for cc in range(4):
                    tail_proj_chunk(i, us[i], cc)

    nc.compile()
    return nc


def _causal_mask(d):
    kp = np.arange(128)[:, None]
    qf = np.arange(QT)[None, :]
    return ((kp + d) <= qf).astype(BF16)


def _prep_inputs(x, w_qkv, b_qkv, w_proj):
    """Build the 8 per-core input maps (host-side shard + pack)."""
    xT = np.ascontiguousarray(x.reshape(T, C).T).astype(BF16)
    mpack = np.concatenate(
        [np.concatenate([_causal_mask(128 * d)] * 2, axis=1)
         for d in range(4)], axis=1)
    ident = np.zeros((128, 64), dtype=BF16)
    ident[np.arange(128), np.arange(128) % 64] = 1
    onesf = np.ones((1, 128), dtype=np.float32)

    def pack_w(wcols):  # [C, 128] -> [128, C] chunk-packed for SBUF
        return np.ascontiguousarray(
            wcols.reshape(8, 128, 128).transpose(1, 0, 2).reshape(128, C)
        ).astype(BF16)

    in_maps = []
    for core in range(NCORES):
        h0 = core * HPC
        cols = slice(h0 * D, (h0 + HPC) * D)  # 128 cols for this core
        wq = pack_w(w_qkv[:, :C][:, cols])
        wk = pack_w(w_qkv[:, C:2 * C][:, cols])
        wv = pack_w(w_qkv[:, 2 * C:][:, cols])
        wp = np.ascontiguousarray(w_proj[cols, :]).astype(BF16)
        wpack = np.concatenate([wq, wk, wv, wp, ident], axis=1)
        spack = np.concatenate(
            [b_qkv[:C][cols].reshape(1, 128),
             b_qkv[C:2 * C][cols].reshape(1, 128),
             b_qkv[2 * C:][cols].reshape(1, 128),
             np.ones((1, QT))], axis=1).astype(BF16)
        m = {
            "xT": xT,
            "wpack": np.ascontiguousarray(wpack),
            "spack": np.ascontiguousarray(spack),
            "mpack": np.ascontiguousarray(mpack),
            "onesf": onesf,
        }
        in_maps.append(m)
    return in_maps


def _get_compiled(with_bias=True):
    if with_bias not in _COMPILED:
        _COMPILED[with_bias] = _build_nc(with_bias=with_bias)
    return _COMPILED[with_bias]


def run_on_device(in_maps, with_bias=True, **kwargs):
    from concourse.bass_utils import run_bass_kernel_spmd

    nc = _get_compiled(with_bias)
    return run_bass_kernel_spmd(nc, in_maps, core_ids=list(range(NCORES)),
                                **kwargs)


def kernel(x, w_qkv, b_qkv, w_proj, b_proj, **run_kwargs):
    x = np.asarray(x, dtype=np.float32)
    w_qkv = np.asarray(w_qkv, dtype=np.float32)
    b_qkv = np.asarray(b_qkv, dtype=np.float32)
    w_proj = np.asarray(w_proj, dtype=np.float32)
    b_proj = np.asarray(b_proj, dtype=np.float32)

    in_maps = _prep_inputs(x, w_qkv, b_qkv, w_proj)
    with_bias = bool(np.any(b_qkv))
    res = run_on_device(in_maps, with_bias=with_bias, **run_kwargs)
    acc = np.zeros((T, C), dtype=np.float32)
    for core in range(NCORES):
        acc += np.asarray(res.results[core]["out"], dtype=np.float32)
    acc += b_proj[None, :]
    out = acc.reshape(1, T, C)
    kernel.last_results = res
    return out
